# revision 25
# baseline (speedup 1.0000x reference)
"""Conv1dLoRA Trainium2 kernel.

Computes, per sample s:
  A[s] = MLP_A(a_emb[s]) in [64, 8]   (Linear-BN-GELU-Linear)
  B[s] = MLP_B(b_emb[s]) in [8, 192]
  W[s] = A[s] @ B[s]  -> per-sample conv weight [cin=64, cout*K=192]
  Y[s] = conv1d(X[s], W[s]*SCALE + base_w, pad=1) + base_b

Sharding: data-parallel over batch. 128 samples -> 16 per core x 8 cores.
MLP/base params are replicated; small host-side numpy transposes put every
weight into the exact SBUF layout the PE needs (no on-device transposes).

The kernel is HBM-DMA-bound (~358 GB/s/core), so all large streams are
bf16: X is cast f32->bf16 on the host before upload (halves the read
stream), Y is produced bf16 on device and cast back to f32 on the host
(halves the write stream). MLP weights upload as bf16 too. This cuts
per-core DMA from ~67 MB to ~35 MB.

Device program (identical SPMD program on all 8 cores):
  - MLPs batched over the 16 local samples; BN+GELU fused into one ACT op.
  - W for a 2-sample pair via ONE K=16 PE matmul: lhsT [16,128] holds
    A[s0].T / A[s1].T block-diagonally, rhs [16,384] holds B[s0] / B[s1]
    with columns pre-permuted tap-major (k*128 + j*64 + cout), so the
    PSUM result is already the block-diagonal conv weight layout; a
    single DVE add folds base_w in (base conv + lora conv = ONE conv).
  - Conv: per 512-col chunk, 3 shifted matmuls (taps) accumulate in PSUM,
    2 samples per matmul via the block-diagonal weights. Zero-padded halo
    columns in SBUF.
  - Bias add fused into the PSUM->SBUF copy (DVE / ACT alternating),
    output written bf16.
"""

import numpy as np
import ml_dtypes

BS, CIN, COUT, L = 128, 64, 64, 8192
K, R, GROUPS = 3, 8, 1
EMB, HID = 256, 256
BN_EPS = 1e-5
NCORES = 8
SH = BS // NCORES          # 16 samples per core
NPAIR = SH // 2            # 8 sample pairs per core
LCH = 512                  # conv chunk (one PSUM bank of fp32)
NCH = L // LCH             # 16 chunks
KCO = K * COUT             # 192 = per-sample W columns (k-major: k*64+cout)

BF16 = ml_dtypes.bfloat16

_NC = None                 # cached compiled Bass program


def _build_program():
    import concourse.tile as tile
    from concourse import bacc, mybir

    f32 = mybir.dt.float32
    bf16 = mybir.dt.bfloat16
    AF = mybir.ActivationFunctionType

    nc = bacc.Bacc(
        "TRN2",
        target_bir_lowering=False,
        debug=False,
        enable_asserts=False,
        num_devices=NCORES,
    )

    def dt_in(name, shape, dt=bf16):
        return nc.dram_tensor(name, shape, dt, kind="ExternalInput").ap()

    X = dt_in("X", [SH, CIN, L + 2])             # bf16, host pre-cast AND
    # host pre-padded with one zero column on each side (conv halo), so the
    # device never touches the halo (no memsets in the DVE queue)
    aT = dt_in("aT", [EMB, SH])
    bT = dt_in("bT", [EMB, SH])
    Aw1T = dt_in("Aw1T", [EMB, HID])             # [e, h]
    Bw1T = dt_in("Bw1T", [EMB, HID])
    Aw2T = dt_in("Aw2T", [HID, CIN * R])         # [h, m'] m' = r*64+i (r-major)
    Bw2T = dt_in("Bw2T", [HID, R * KCO])         # [h, m'] m' = r*192 + k*64 + cout
    vecs = dt_in("vecs", [128, 9], f32)          # gA0 gA1 cA0 cA1 gB0 gB1 cB0 cB1 bias
    b2A = dt_in("b2A", [1, CIN * R])             # layer-2 bias rows (permuted)
    b2B = dt_in("b2B", [1, R * KCO])
    base_pair = dt_in("base_pair", [128, 2 * KCO], f32)  # tap-major block-diag base_w
    Y = nc.dram_tensor("Y", [SH, COUT, L], bf16, kind="ExternalOutput").ap()

    with tile.TileContext(nc) as tc:
        with (
            tc.tile_pool(name="const", bufs=1) as const,
            # conv-phase pools opened first so their SBUF/PSUM addresses are
            # never reused from transient pools (address reuse would add
            # write-after-read deps that stall the conv stream)
            tc.tile_pool(name="yps", bufs=3, space="PSUM") as yps,
            tc.tile_pool(name="xpool", bufs=6) as xpool,
            tc.tile_pool(name="ypool", bufs=6) as ypool,
            tc.tile_pool(name="wpool", bufs=NPAIR) as wpool,
            tc.tile_pool(name="wps", bufs=2, space="PSUM") as wps,
        ):
            # ---- constants; MLP-critical ones first. X loads own the sync
            # ---- queue from t=0; consts go on scalar/gpsimd queues.
            def load(name, src_ap, shape, dt=bf16, eng=None):
                t = const.tile(list(shape), dt, name=name, tag=name)
                (eng or nc.scalar).dma_start(t[:], src_ap)
                return t

            aT_sb = [load(f"aT{e}", aT[e * 128:(e + 1) * 128], (128, SH)) for e in range(2)]
            bT_sb = [load(f"bT{e}", bT[e * 128:(e + 1) * 128], (128, SH)) for e in range(2)]
            vecs_sb = load("vecs", vecs, (128, 9), dt=f32)
            gA_sb = [vecs_sb[:, h:h + 1] for h in range(2)]
            cA_sb = [vecs_sb[:, 2 + h:3 + h] for h in range(2)]
            gB_sb = [vecs_sb[:, 4 + h:5 + h] for h in range(2)]
            cB_sb = [vecs_sb[:, 6 + h:7 + h] for h in range(2)]
            bias_sb = vecs_sb[:, 8:9]
            Aw1T_sb = [load(f"Aw1T{e}", Aw1T[e * 128:(e + 1) * 128], (128, HID)) for e in range(2)]
            Bw1T_sb = [load(f"Bw1T{e}", Bw1T[e * 128:(e + 1) * 128], (128, HID)) for e in range(2)]
            Aw2T_sb = [load(f"Aw2T{h}", Aw2T[h * 128:(h + 1) * 128], (128, CIN * R)) for h in range(2)]
            Bw2T_sb = [load(f"Bw2T{h}", Bw2T[h * 128:(h + 1) * 128], (128, R * KCO)) for h in range(2)]
            b2A_sb = load("b2A", b2A, (1, CIN * R))
            b2B_sb = load("b2B", b2B, (1, R * KCO))
            base_sb = load("base_pair", base_pair, (128, 2 * KCO), dt=f32)
            ones_sb = const.tile([1, SH], bf16, name="ones", tag="ones")
            nc.vector.memset(ones_sb[:], 1.0)

            A_row = const.tile([SH, CIN * R], bf16, name="A_row", tag="A_row")
            B_row = const.tile([SH, R * KCO], bf16, name="B_row", tag="B_row")

            # W-gen staging: three rotating sets of block-diagonal tiles
            # (W-gen runs three pairs ahead of the conv stream).
            # ast[j*8+r, j*64+i] = A[s_{2t+j}][i, r]; off-diag stays zero.
            # bst[j*8+r, k*128+j*64+c] = B[s_{2t+j}][r, k*64+c]; rest zero.
            NW = 3
            ast_sb, bst_sb = [], []
            for v in range(NW):
                a = const.tile([2 * R, 2 * CIN], bf16, name=f"ast{v}", tag=f"ast{v}")
                b = const.tile([2 * R, 2 * KCO], bf16, name=f"bst{v}", tag=f"bst{v}")
                nc.vector.memset(a[:], 0.0)
                nc.vector.memset(b[:], 0.0)
                ast_sb.append(a)
                bst_sb.append(b)

            # ---- MLPs (batched over the 16 local samples); PSUM shares the
            # ---- wps pool slots (tag "psw"), recycled before W-gen needs them
            gel = {}
            for side, w1T, embT, g_sb, c_sb in (
                ("A", Aw1T_sb, aT_sb, gA_sb, cA_sb),
                ("B", Bw1T_sb, bT_sb, gB_sb, cB_sb),
            ):
                for hc in range(2):
                    ps1 = wps.tile([128, SH], f32, name=f"ps1{side}{hc}", tag="psw")
                    for ec in range(2):
                        nc.tensor.matmul(
                            ps1[:],
                            w1T[ec][:, hc * 128:(hc + 1) * 128],
                            embT[ec][:],
                            start=(ec == 0),
                            stop=(ec == 1),
                        )
                    g = const.tile([128, SH], bf16, name=f"gel{side}{hc}", tag=f"gel{side}{hc}")
                    # gelu(h * g' + (b1*g' + beta)) == BN+bias+GELU fused
                    nc.scalar.activation(
                        g[:], ps1[:], AF.Gelu, bias=c_sb[hc][:], scale=g_sb[hc][:]
                    )
                    gel[(side, hc)] = g

            for side, w2T_sb, b2_sb, dst, width in (
                ("A", Aw2T_sb, b2A_sb, A_row, CIN * R),
                ("B", Bw2T_sb, b2B_sb, B_row, R * KCO),
            ):
                for nb in range(width // 512):
                    ps2 = wps.tile([SH, 512], f32, name=f"ps2{side}{nb}", tag="psw")
                    for hc in range(2):
                        nc.tensor.matmul(
                            ps2[:],
                            gel[(side, hc)][:],
                            w2T_sb[hc][:, nb * 512:(nb + 1) * 512],
                            start=(hc == 0),
                            stop=False,
                        )
                    # + layer-2 bias via rank-1 ones matmul
                    nc.tensor.matmul(
                        ps2[:],
                        ones_sb[:],
                        b2_sb[:, nb * 512:(nb + 1) * 512],
                        start=False,
                        stop=True,
                    )
                    nc.vector.tensor_copy(dst[:, nb * 512:(nb + 1) * 512], ps2[:])

            # ---- conv stream; W generation software-pipelined one pair
            # ---- ahead so its DVE repack never queues behind the PSUM
            # ---- evacuation copies of the current pair
            OB = 4096                      # output block columns (8KB descs)

            def emit_w(t):
                ast, bst = ast_sb[t % NW], bst_sb[t % NW]
                for j in range(2):
                    s = 2 * t + j
                    # A_row[s] is [r(8), i(64)] r-major; DMA reshapes the one
                    # source row onto 8 partitions (flat byte order matches)
                    nc.gpsimd.dma_start(
                        ast[j * R:(j + 1) * R, j * CIN:(j + 1) * CIN],
                        A_row[s:s + 1, :],
                    )
                    # B_row[s] is [r(8), k(3), c(64)]; dst cols split per tap
                    # at k*128 + j*64
                    nc.gpsimd.dma_start(
                        bst[j * R:(j + 1) * R, :]
                        .rearrange("p (k jj c) -> p k jj c", k=K, jj=2)[:, :, j, :],
                        B_row[s:s + 1, :],
                    )
                # W pair = ast.T @ bst: one K=16 matmul; off-diagonal zeros
                # in ast/bst keep the cross-sample blocks zero, so psw is
                # the tap-major block-diagonal weight layout directly.
                psw = wps.tile([128, 2 * KCO], f32, name=f"psw{t}", tag="psw")
                nc.tensor.matmul(psw[:], ast[:], bst[:], start=True, stop=True)
                wpk = wpool.tile([128, 2 * KCO], bf16, name=f"wpk{t}", tag="wpk")
                nc.vector.tensor_add(wpk[:], psw[:], base_sb[:])
                return wpk

            wpks = [emit_w(0), emit_w(1), emit_w(2)]
            for t in range(NPAIR):
                wpk = wpks[t]
                if t + NW < NPAIR:
                    wpks.append(emit_w(t + NW))
                xp = xpool.tile([128, L + 2], bf16, name=f"xp{t}", tag="xp")
                # X load split in two halves (8KB descs) on the sync HWDGE
                # queue, so the first conv chunks start after half a load and
                # load/store transfers interleave at ~3us granularity
                xh = (L + 2) // 2
                nc.sync.dma_start(xp[:, :xh], X[2 * t:2 * t + 2, :, :xh])
                nc.sync.dma_start(xp[:, xh:], X[2 * t:2 * t + 2, :, xh:])
                for ob in range(L // OB):
                    yo = ypool.tile([128, OB], bf16, name=f"yo{t}_{ob}", tag="yo")
                    for h2 in range(OB // (2 * LCH)):
                        # two chunks share one 2-bank PSUM tile so each
                        # evacuation copy covers 1024 cols (half the copies)
                        yp2 = yps.tile([128, 2 * LCH], f32, name=f"yp{t}_{ob}_{h2}", tag="yp")
                        for half in range(2):
                            c = (ob * (OB // LCH)) + h2 * 2 + half
                            for k in range(K):
                                nc.tensor.matmul(
                                    yp2[:, half * LCH:(half + 1) * LCH],
                                    wpk[:, k * 128:(k + 1) * 128],
                                    xp[:, c * LCH + k:c * LCH + k + LCH],
                                    start=(k == 0),
                                    stop=(k == K - 1),
                                )
                        # bias fused into the PSUM->SBUF copy, alternating
                        # DVE / ACT so neither engine is the bottleneck
                        dst = yo[:, h2 * 2 * LCH:(h2 + 1) * 2 * LCH]
                        if h2 % 2 == 0:
                            nc.vector.tensor_scalar_add(dst, yp2[:], bias_sb[:])
                        else:
                            nc.scalar.activation(
                                dst, yp2[:], AF.Identity, bias=bias_sb[:]
                            )
                    # one DMA per output block (3D AP covers both samples) on
                    # the otherwise-idle gpsimd SWDGE queue, so store waits
                    # never block the ACT/DVE copy streams
                    lo, hi = ob * OB, (ob + 1) * OB
                    nc.gpsimd.dma_start(Y[2 * t:2 * t + 2, :, lo:hi], yo[:])

    nc.compile()
    return nc


def _host_prep(inputs):
    """Shared (replicated) tensors, in device layouts. Returns dict of np arrays."""
    f = np.float32
    gA_flat = (inputs["A_bn_g"] / np.sqrt(f(1.0) + f(BN_EPS))).astype(f)
    gB_flat = (inputs["B_bn_g"] / np.sqrt(f(1.0) + f(BN_EPS))).astype(f)
    cA_flat = (inputs["A_b1"] * gA_flat + inputs["A_bn_b"]).astype(f)
    cB_flat = (inputs["B_b1"] * gB_flat + inputs["B_bn_b"]).astype(f)

    # A layer-2: columns m = i*8+r  ->  m' = r*64+i (r-major)
    permA = (np.arange(R)[:, None] + np.arange(CIN)[None, :] * R).reshape(-1)  # m'[r,i] -> i*8+r
    Aw2T = np.ascontiguousarray(inputs["A_w2"].T[:, permA]).astype(BF16)
    b2A = np.ascontiguousarray(inputs["A_b2"][permA]).astype(BF16).reshape(1, CIN * R)

    # B layer-2: columns m = r*192 + cout*3 + k  ->  m' = r*192 + k*64 + cout
    m2 = (np.arange(COUT)[None, :] * K + np.arange(K)[:, None]).reshape(-1)  # m2'[k,c] -> c*3+k
    permB = (np.arange(R)[:, None] * KCO + m2[None, :]).reshape(-1)
    Bw2T = np.ascontiguousarray(inputs["B_w2"].T[:, permB]).astype(BF16)
    b2B = np.ascontiguousarray(inputs["B_b2"][permB]).astype(BF16).reshape(1, R * KCO)

    # base_w [cout, cin, k] -> tap-major block-diag pair layout:
    # base_pair[j*64 + i, k*128 + j*64 + c] = base_w[c, i, k]
    base_pair = np.zeros((128, 2 * KCO), dtype=f)
    for j in range(2):
        for k in range(K):
            base_pair[j * 64:(j + 1) * 64, k * 128 + j * 64:k * 128 + j * 64 + 64] = (
                inputs["base_w"][:, :, k].T.astype(f)
            )

    bias_out = np.concatenate([inputs["base_b"], inputs["base_b"]]).astype(f)

    # all per-partition vectors in one tensor -> one early DMA:
    # cols = gA0 gA1 cA0 cA1 gB0 gB1 cB0 cB1 bias_out
    vecs = np.stack([
        gA_flat[:128], gA_flat[128:], cA_flat[:128], cA_flat[128:],
        gB_flat[:128], gB_flat[128:], cB_flat[:128], cB_flat[128:],
        bias_out,
    ], axis=1).astype(f)

    return {
        "Aw1T": np.ascontiguousarray(inputs["A_w1"].T).astype(BF16),
        "Bw1T": np.ascontiguousarray(inputs["B_w1"].T).astype(BF16),
        "Aw2T": Aw2T,
        "Bw2T": Bw2T,
        "vecs": vecs,
        "b2A": b2A,
        "b2B": b2B,
        "base_pair": base_pair,
    }


def _in_maps(inputs):
    shared = _host_prep(inputs)
    maps = []
    for c in range(NCORES):
        lo, hi = c * SH, (c + 1) * SH
        m = dict(shared)
        xp = np.zeros((SH, CIN, L + 2), dtype=BF16)
        xp[:, :, 1:L + 1] = inputs["X"][lo:hi].astype(BF16)
        m["X"] = xp
        m["aT"] = np.ascontiguousarray(inputs["a_embedding"][lo:hi].T).astype(BF16)
        m["bT"] = np.ascontiguousarray(inputs["b_embedding"][lo:hi].T).astype(BF16)
        maps.append(m)
    return maps


def run(inputs, trace=False):
    """Run the kernel; returns (Y_full, BassKernelResults)."""
    global _NC
    if _NC is None:
        _NC = _build_program()
    from concourse.bass_utils import run_bass_kernel_spmd

    res = run_bass_kernel_spmd(
        _NC, _in_maps(inputs), core_ids=list(range(NCORES)), trace=trace
    )
    Y = np.concatenate([r["Y"] for r in res.results], axis=0).astype(np.float32)
    return Y, res


def kernel(**inputs) -> np.ndarray:
    Y, _ = run(inputs, trace=False)
    return Y


# revision 29
# speedup vs baseline: 1.1086x; 1.1086x over previous
"""Conv1dLoRA Trainium2 kernel.

Computes, per sample s:
  A[s] = MLP_A(a_emb[s]) in [64, 8]   (Linear-BN-GELU-Linear)
  B[s] = MLP_B(b_emb[s]) in [8, 192]
  W[s] = A[s] @ B[s]  -> per-sample conv weight [cin=64, cout*K=192]
  Y[s] = conv1d(X[s], W[s]*SCALE + base_w, pad=1) + base_b

Sharding: data-parallel over batch. 128 samples -> 16 per core x 8 cores.
MLP/base params are replicated; small host-side numpy transposes put every
weight into the exact SBUF layout the PE needs (no on-device transposes).

The kernel is HBM-DMA-bound (~358 GB/s/core), so all large streams are
bf16: X is cast f32->bf16 on the host before upload (halves the read
stream), Y is produced bf16 on device and cast back to f32 on the host
(halves the write stream). MLP weights upload as bf16 too. This cuts
per-core DMA from ~67 MB to ~35 MB.

Device program (identical SPMD program on all 8 cores):
  - MLPs batched over the 16 local samples; BN+GELU fused into one ACT op.
  - W for a 2-sample pair via ONE K=16 PE matmul: lhsT [16,128] holds
    A[s0].T / A[s1].T block-diagonally, rhs [16,384] holds B[s0] / B[s1]
    with columns pre-permuted tap-major (k*128 + j*64 + cout), so the
    PSUM result is already the block-diagonal conv weight layout; a
    single DVE add folds base_w in (base conv + lora conv = ONE conv).
  - Conv: per 512-col chunk, 3 shifted matmuls (taps) accumulate in PSUM,
    2 samples per matmul via the block-diagonal weights. Zero-padded halo
    columns in SBUF.
  - Bias add fused into the PSUM->SBUF copy (DVE / ACT alternating),
    output written bf16.
"""

import numpy as np
import ml_dtypes

BS, CIN, COUT, L = 128, 64, 64, 8192
K, R, GROUPS = 3, 8, 1
EMB, HID = 256, 256
BN_EPS = 1e-5
NCORES = 8
SH = BS // NCORES          # 16 samples per core
NPAIR = SH // 2            # 8 sample pairs per core
LCH = 512                  # conv chunk (one PSUM bank of fp32)
NCH = L // LCH             # 16 chunks
KCO = K * COUT             # 192 = per-sample W columns (k-major: k*64+cout)

BF16 = ml_dtypes.bfloat16

_NC = None                 # cached compiled Bass program


def _build_program():
    import concourse.tile as tile
    from concourse import bacc, mybir

    f32 = mybir.dt.float32
    bf16 = mybir.dt.bfloat16
    AF = mybir.ActivationFunctionType

    nc = bacc.Bacc(
        "TRN2",
        target_bir_lowering=False,
        debug=False,
        enable_asserts=False,
        num_devices=NCORES,
    )

    def dt_in(name, shape, dt=bf16):
        return nc.dram_tensor(name, shape, dt, kind="ExternalInput").ap()

    X = dt_in("X", [SH, CIN, L + 2])             # bf16, host pre-cast AND
    # host pre-padded with one zero column on each side (conv halo), so the
    # device never touches the halo (no memsets in the DVE queue)
    aT = dt_in("aT", [EMB, SH])
    bT = dt_in("bT", [EMB, SH])
    Aw1T = dt_in("Aw1T", [EMB, HID])             # [e, h]
    Bw1T = dt_in("Bw1T", [EMB, HID])
    Aw2T = dt_in("Aw2T", [HID, CIN * R])         # [h, m'] m' = r*64+i (r-major)
    Bw2T = dt_in("Bw2T", [HID, R * KCO])         # [h, m'] m' = r*192 + k*64 + cout
    vecs = dt_in("vecs", [128, 9], f32)          # gA0 gA1 cA0 cA1 gB0 gB1 cB0 cB1 bias
    b2A = dt_in("b2A", [1, CIN * R])             # layer-2 bias rows (permuted)
    b2B = dt_in("b2B", [1, R * KCO])
    base_pair = dt_in("base_pair", [128, 2 * KCO], f32)  # tap-major block-diag base_w
    Y = nc.dram_tensor("Y", [SH, COUT, L], bf16, kind="ExternalOutput").ap()

    with tile.TileContext(nc) as tc:
        with (
            tc.tile_pool(name="const", bufs=1) as const,
            # conv-phase pools opened first so their SBUF/PSUM addresses are
            # never reused from transient pools (address reuse would add
            # write-after-read deps that stall the conv stream)
            tc.tile_pool(name="yps", bufs=3, space="PSUM") as yps,
            tc.tile_pool(name="xpool", bufs=6) as xpool,
            tc.tile_pool(name="ypool", bufs=6) as ypool,
            tc.tile_pool(name="wpool", bufs=NPAIR) as wpool,
            tc.tile_pool(name="wps", bufs=2, space="PSUM") as wps,
        ):
            # ---- constants; MLP-critical ones first, all on the sync queue
            # ---- AHEAD of the X loads (same-queue FIFO guarantees the
            # ---- consts beat the X prefetch burst to the DMA engines)
            def load(name, src_ap, shape, dt=bf16, eng=None):
                t = const.tile(list(shape), dt, name=name, tag=name)
                (eng or nc.sync).dma_start(t[:], src_ap)
                return t

            aT_sb = [load(f"aT{e}", aT[e * 128:(e + 1) * 128], (128, SH)) for e in range(2)]
            bT_sb = [load(f"bT{e}", bT[e * 128:(e + 1) * 128], (128, SH)) for e in range(2)]
            vecs_sb = load("vecs", vecs, (128, 9), dt=f32)
            gA_sb = [vecs_sb[:, h:h + 1] for h in range(2)]
            cA_sb = [vecs_sb[:, 2 + h:3 + h] for h in range(2)]
            gB_sb = [vecs_sb[:, 4 + h:5 + h] for h in range(2)]
            cB_sb = [vecs_sb[:, 6 + h:7 + h] for h in range(2)]
            bias_sb = vecs_sb[:, 8:9]
            Aw1T_sb = [load(f"Aw1T{e}", Aw1T[e * 128:(e + 1) * 128], (128, HID)) for e in range(2)]
            Bw1T_sb = [load(f"Bw1T{e}", Bw1T[e * 128:(e + 1) * 128], (128, HID)) for e in range(2)]
            Aw2T_sb = [load(f"Aw2T{h}", Aw2T[h * 128:(h + 1) * 128], (128, CIN * R)) for h in range(2)]
            Bw2T_sb = [load(f"Bw2T{h}", Bw2T[h * 128:(h + 1) * 128], (128, R * KCO)) for h in range(2)]
            b2A_sb = load("b2A", b2A, (1, CIN * R))
            b2B_sb = load("b2B", b2B, (1, R * KCO))
            base_sb = load("base_pair", base_pair, (128, 2 * KCO), dt=f32)
            ones_sb = const.tile([1, SH], bf16, name="ones", tag="ones")
            nc.vector.memset(ones_sb[:], 1.0)

            A_row = const.tile([SH, CIN * R], bf16, name="A_row", tag="A_row")
            B_row = const.tile([SH, R * KCO], bf16, name="B_row", tag="B_row")

            # W-gen staging: two alternating sets of block-diagonal tiles
            # (W-gen runs one pair ahead of the conv stream).
            # ast[j*8+r, j*64+i] = A[s_{2t+j}][i, r]; off-diag stays zero.
            # bst[j*8+r, k*128+j*64+c] = B[s_{2t+j}][r, k*64+c]; rest zero.
            NW = 2
            ast_sb, bst_sb = [], []
            for v in range(NW):
                a = const.tile([2 * R, 2 * CIN], bf16, name=f"ast{v}", tag=f"ast{v}")
                b = const.tile([2 * R, 2 * KCO], bf16, name=f"bst{v}", tag=f"bst{v}")
                nc.vector.memset(a[:], 0.0)
                nc.vector.memset(b[:], 0.0)
                ast_sb.append(a)
                bst_sb.append(b)

            # ---- MLPs (batched over the 16 local samples); PSUM shares the
            # ---- wps pool slots (tag "psw"), recycled before W-gen needs them
            gel = {}
            for side, w1T, embT, g_sb, c_sb in (
                ("A", Aw1T_sb, aT_sb, gA_sb, cA_sb),
                ("B", Bw1T_sb, bT_sb, gB_sb, cB_sb),
            ):
                for hc in range(2):
                    ps1 = wps.tile([128, SH], f32, name=f"ps1{side}{hc}", tag="psw")
                    for ec in range(2):
                        nc.tensor.matmul(
                            ps1[:],
                            w1T[ec][:, hc * 128:(hc + 1) * 128],
                            embT[ec][:],
                            start=(ec == 0),
                            stop=(ec == 1),
                        )
                    g = const.tile([128, SH], bf16, name=f"gel{side}{hc}", tag=f"gel{side}{hc}")
                    # gelu(h * g' + (b1*g' + beta)) == BN+bias+GELU fused
                    nc.scalar.activation(
                        g[:], ps1[:], AF.Gelu, bias=c_sb[hc][:], scale=g_sb[hc][:]
                    )
                    gel[(side, hc)] = g

            for side, w2T_sb, b2_sb, dst, width in (
                ("A", Aw2T_sb, b2A_sb, A_row, CIN * R),
                ("B", Bw2T_sb, b2B_sb, B_row, R * KCO),
            ):
                for nb in range(width // 512):
                    ps2 = wps.tile([SH, 512], f32, name=f"ps2{side}{nb}", tag="psw")
                    for hc in range(2):
                        nc.tensor.matmul(
                            ps2[:],
                            gel[(side, hc)][:],
                            w2T_sb[hc][:, nb * 512:(nb + 1) * 512],
                            start=(hc == 0),
                            stop=False,
                        )
                    # + layer-2 bias via rank-1 ones matmul
                    nc.tensor.matmul(
                        ps2[:],
                        ones_sb[:],
                        b2_sb[:, nb * 512:(nb + 1) * 512],
                        start=False,
                        stop=True,
                    )
                    nc.vector.tensor_copy(dst[:, nb * 512:(nb + 1) * 512], ps2[:])

            # ---- conv stream; W generation software-pipelined one pair
            # ---- ahead so its DVE repack never queues behind the PSUM
            # ---- evacuation copies of the current pair
            OB = 4096                      # output block columns (8KB descs)

            def emit_w(t):
                ast, bst = ast_sb[t % NW], bst_sb[t % NW]
                for j in range(2):
                    s = 2 * t + j
                    # staging rides the scalar queue just ahead of the
                    # PE-paced ACT copies (never behind Y stores); its sem
                    # waits are always already satisfied so it cannot
                    # head-of-line block the copies.
                    # A_row[s] is [r(8), i(64)] r-major; DMA reshapes the one
                    # source row onto 8 partitions (flat byte order matches)
                    nc.scalar.dma_start(
                        ast[j * R:(j + 1) * R, j * CIN:(j + 1) * CIN],
                        A_row[s:s + 1, :],
                    )
                    # B_row[s] is [r(8), k(3), c(64)]; dst cols split per tap
                    # at k*128 + j*64
                    nc.scalar.dma_start(
                        bst[j * R:(j + 1) * R, :]
                        .rearrange("p (k jj c) -> p k jj c", k=K, jj=2)[:, :, j, :],
                        B_row[s:s + 1, :],
                    )
                # W pair = ast.T @ bst: one K=16 matmul; off-diagonal zeros
                # in ast/bst keep the cross-sample blocks zero, so psw is
                # the tap-major block-diagonal weight layout directly.
                psw = wps.tile([128, 2 * KCO], f32, name=f"psw{t}", tag="psw")
                nc.tensor.matmul(psw[:], ast[:], bst[:], start=True, stop=True)
                wpk = wpool.tile([128, 2 * KCO], bf16, name=f"wpk{t}", tag="wpk")
                nc.vector.tensor_add(wpk[:], psw[:], base_sb[:])
                return wpk

            wpks = [emit_w(0)]
            for t in range(NPAIR):
                wpk = wpks[t]
                if t + 1 < NPAIR:
                    wpks.append(emit_w(t + 1))
                xp = xpool.tile([128, L + 2], bf16, name=f"xp{t}", tag="xp")
                # X load split in two halves (8KB descs) on the sync HWDGE
                # queue, so the first conv chunks start after half a load and
                # load/store transfers interleave at ~3us granularity
                xh = (L + 2) // 2
                nc.sync.dma_start(xp[:, :xh], X[2 * t:2 * t + 2, :, :xh])
                nc.sync.dma_start(xp[:, xh:], X[2 * t:2 * t + 2, :, xh:])
                for ob in range(L // OB):
                    yo = ypool.tile([128, OB], bf16, name=f"yo{t}_{ob}", tag="yo")
                    for h2 in range(OB // (2 * LCH)):
                        # two chunks share one 2-bank PSUM tile so each
                        # evacuation copy covers 1024 cols (half the copies)
                        yp2 = yps.tile([128, 2 * LCH], f32, name=f"yp{t}_{ob}_{h2}", tag="yp")
                        for half in range(2):
                            c = (ob * (OB // LCH)) + h2 * 2 + half
                            for k in range(K):
                                nc.tensor.matmul(
                                    yp2[:, half * LCH:(half + 1) * LCH],
                                    wpk[:, k * 128:(k + 1) * 128],
                                    xp[:, c * LCH + k:c * LCH + k + LCH],
                                    start=(k == 0),
                                    stop=(k == K - 1),
                                )
                        # bias fused into the PSUM->SBUF copy, alternating
                        # DVE / ACT so neither engine is the bottleneck
                        dst = yo[:, h2 * 2 * LCH:(h2 + 1) * 2 * LCH]
                        if h2 % 2 == 0:
                            nc.vector.tensor_scalar_add(dst, yp2[:], bias_sb[:])
                        else:
                            nc.scalar.activation(
                                dst, yp2[:], AF.Identity, bias=bias_sb[:]
                            )
                    # one DMA per output block (3D AP covers both samples) on
                    # the otherwise-idle gpsimd SWDGE queue, so store waits
                    # never block the ACT/DVE copy streams
                    lo, hi = ob * OB, (ob + 1) * OB
                    nc.gpsimd.dma_start(Y[2 * t:2 * t + 2, :, lo:hi], yo[:])

    nc.compile()
    return nc


def _host_prep(inputs):
    """Shared (replicated) tensors, in device layouts. Returns dict of np arrays."""
    f = np.float32
    gA_flat = (inputs["A_bn_g"] / np.sqrt(f(1.0) + f(BN_EPS))).astype(f)
    gB_flat = (inputs["B_bn_g"] / np.sqrt(f(1.0) + f(BN_EPS))).astype(f)
    cA_flat = (inputs["A_b1"] * gA_flat + inputs["A_bn_b"]).astype(f)
    cB_flat = (inputs["B_b1"] * gB_flat + inputs["B_bn_b"]).astype(f)

    # A layer-2: columns m = i*8+r  ->  m' = r*64+i (r-major)
    permA = (np.arange(R)[:, None] + np.arange(CIN)[None, :] * R).reshape(-1)  # m'[r,i] -> i*8+r
    Aw2T = np.ascontiguousarray(inputs["A_w2"].T[:, permA]).astype(BF16)
    b2A = np.ascontiguousarray(inputs["A_b2"][permA]).astype(BF16).reshape(1, CIN * R)

    # B layer-2: columns m = r*192 + cout*3 + k  ->  m' = r*192 + k*64 + cout
    m2 = (np.arange(COUT)[None, :] * K + np.arange(K)[:, None]).reshape(-1)  # m2'[k,c] -> c*3+k
    permB = (np.arange(R)[:, None] * KCO + m2[None, :]).reshape(-1)
    Bw2T = np.ascontiguousarray(inputs["B_w2"].T[:, permB]).astype(BF16)
    b2B = np.ascontiguousarray(inputs["B_b2"][permB]).astype(BF16).reshape(1, R * KCO)

    # base_w [cout, cin, k] -> tap-major block-diag pair layout:
    # base_pair[j*64 + i, k*128 + j*64 + c] = base_w[c, i, k]
    base_pair = np.zeros((128, 2 * KCO), dtype=f)
    for j in range(2):
        for k in range(K):
            base_pair[j * 64:(j + 1) * 64, k * 128 + j * 64:k * 128 + j * 64 + 64] = (
                inputs["base_w"][:, :, k].T.astype(f)
            )

    bias_out = np.concatenate([inputs["base_b"], inputs["base_b"]]).astype(f)

    # all per-partition vectors in one tensor -> one early DMA:
    # cols = gA0 gA1 cA0 cA1 gB0 gB1 cB0 cB1 bias_out
    vecs = np.stack([
        gA_flat[:128], gA_flat[128:], cA_flat[:128], cA_flat[128:],
        gB_flat[:128], gB_flat[128:], cB_flat[:128], cB_flat[128:],
        bias_out,
    ], axis=1).astype(f)

    return {
        "Aw1T": np.ascontiguousarray(inputs["A_w1"].T).astype(BF16),
        "Bw1T": np.ascontiguousarray(inputs["B_w1"].T).astype(BF16),
        "Aw2T": Aw2T,
        "Bw2T": Bw2T,
        "vecs": vecs,
        "b2A": b2A,
        "b2B": b2B,
        "base_pair": base_pair,
    }


def _in_maps(inputs):
    shared = _host_prep(inputs)
    maps = []
    for c in range(NCORES):
        lo, hi = c * SH, (c + 1) * SH
        m = dict(shared)
        xp = np.zeros((SH, CIN, L + 2), dtype=BF16)
        xp[:, :, 1:L + 1] = inputs["X"][lo:hi].astype(BF16)
        m["X"] = xp
        m["aT"] = np.ascontiguousarray(inputs["a_embedding"][lo:hi].T).astype(BF16)
        m["bT"] = np.ascontiguousarray(inputs["b_embedding"][lo:hi].T).astype(BF16)
        maps.append(m)
    return maps


def run(inputs, trace=False):
    """Run the kernel; returns (Y_full, BassKernelResults)."""
    global _NC
    if _NC is None:
        _NC = _build_program()
    from concourse.bass_utils import run_bass_kernel_spmd

    res = run_bass_kernel_spmd(
        _NC, _in_maps(inputs), core_ids=list(range(NCORES)), trace=trace
    )
    Y = np.concatenate([r["Y"] for r in res.results], axis=0).astype(np.float32)
    return Y, res


def kernel(**inputs) -> np.ndarray:
    Y, _ = run(inputs, trace=False)
    return Y
